# revision 11
# baseline (speedup 1.0000x reference)
"""Trainium2 Bass kernel: 12-head attention block (qkv proj -> softmax attn -> fc).

Reference semantics (B=32, S=577, D=768, H=12, Dh=64):
    qkv = x @ w_qkv + b_qkv
    q, k, v = split(qkv); attn = softmax(q k^T / 8) v
    out = attn @ w_fc + b_fc

Sharding: data-parallel over batch across 8 NeuronCores (4 images per core),
weights replicated, no collectives. Compute in bf16 with fp32 PSUM accumulation.

Layout strategy per core (all matmuls contract over the partition dim):
  - xT [768, 577] built from x via PE transposes (fp32 transpose mode).
  - qkT [1536, 577] = w_qkv[:, :1536]^T . xT  (w_qkv stationary in natural layout).
  - v   [577, 768]  = xT^T . w_qkv[:, 1536:]  (natural layout, per-head ones column
    appended so attention row-sums fall out of the attn@v matmul for free).
  - scoresT[sk, sq] = kT_h^T . qT_h; heads paired even/odd so their K=64
    matmuls land on disjoint PE row groups and run concurrently. exp on
    ScalarE (scale=1/8 folded into exp; no max subtraction -- scores are O(1)).
  - attn_outT[65, sq] = (v_h|1)^T . expT ; row 64 = softmax denominators.
  - normalize: reciprocal_approx_fast straight off the PSUM sum row, gpsimd
    partition_broadcast, then ONE fused DVE multiply (po x rbc -> attnT bf16)
    that is also the PSUM drain (no separate copy).
  - qkT drain is a DVE tensor_scalar_add (per-partition bias), NOT a ScalarE
    activation: ScalarE is the attention pacer (2x 774ns exp per si-step) and
    must not carry the bias adds too.
  - fc: out[s, :] = attn_T_k^T . w_fc_k (natural w_fc), + b_fc broadcast.

Scheduling (v2, trace-driven): the HAM clock gate re-throttled the PE to half
clock for ~105us in v1 (startup, batch boundaries, last batch) whenever the MM
supply thinned. Fixes:
  - ~10 junk warmup matmuls at t~0 so HAM unthrottles during the DMA-bound
    startup instead of 30us in.
  - all non-attention PE work is cut into ~1-2us filler units (qkT per m-tile,
    v per si-tile, fc per si-tile, xT per dk) and fed into the attention
    si-step loop with an even budget (queue_total / 30 steps per batch), so
    the MM stream never thins at batch boundaries.
  - attnv heads are woven INTO the next pair's scores si-loop (head A after
    si1, head B after si3) instead of running as a block after it.
  - batch 3 (which has no next batch to prefetch) gets v(3) moved into its own
    attention as its leading fillers, plus fc(2), so its MM supply matches the
    other batches.
PSUM: two 2-slot pools (scores vs everything else), 8 banks total.
"""

import os
import sys

import numpy as np

for _p in ("/opt/trn_rl_repo", "/root/.axon_site/_ro/trn_rl_repo"):
    if os.path.isdir(_p) and _p not in sys.path:
        sys.path.insert(0, _p)

import concourse.bass as bass  # noqa: E402
import concourse.tile as tile  # noqa: E402
from concourse import bacc, mybir  # noqa: E402
from concourse.bass_utils import run_bass_kernel_spmd  # noqa: E402
from concourse.masks import make_identity  # noqa: E402

F32 = mybir.dt.float32
BF16 = mybir.dt.bfloat16

B, S, D = 32, 577, 768
H, DH = 12, 64
NCORES = 8
NB = B // NCORES  # 4 batch images per core
SCALE = DH**-0.5  # 0.125
NKT = D // 128  # 6 contraction tiles of 128
S_TILES = [(0, 128), (128, 128), (256, 128), (384, 128), (512, 65)]
CH_S = [(0, 512), (512, 65)]  # 577 split at PSUM-bank boundary
CH_D = [(0, 512), (512, 256)]  # 768 split at PSUM-bank boundary
EXP = mybir.ActivationFunctionType.Exp

# filler unit cost estimates (ns of PE stream time, warm)
COST_QKT = 1500
COST_V_SI = 2000
COST_FC_SI = 2000
COST_XT_DK = 900


def build_nc():
    nc = bacc.Bacc(None)
    x_ext = nc.declare_dram_parameter("x", [NB, S, D], F32, isOutput=False)
    wqkv_ext = nc.declare_dram_parameter("w_qkv", [D, 3 * D], F32, isOutput=False)
    bqkv_ext = nc.declare_dram_parameter("b_qkv", [3 * D], F32, isOutput=False)
    wfc_ext = nc.declare_dram_parameter("w_fc", [D, D], F32, isOutput=False)
    bfc_ext = nc.declare_dram_parameter("b_fc", [D], F32, isOutput=False)
    out_ext = nc.declare_dram_parameter("out", [NB, S, D], F32, isOutput=True)

    with tile.TileContext(nc) as tc:
        with (
            tc.tile_pool(name="const", bufs=1) as cpool,
            tc.tile_pool(name="x", bufs=2) as x_pool,
            tc.tile_pool(name="xT", bufs=2) as xT_pool,
            tc.tile_pool(name="qkT", bufs=2) as qkT_pool,
            tc.tile_pool(name="v", bufs=2) as v_pool,
            tc.tile_pool(name="expT", bufs=5) as expT_pool,
            tc.tile_pool(name="attnT", bufs=2) as attnT_pool,
            tc.tile_pool(name="small", bufs=3) as small_pool,
            tc.tile_pool(name="osb", bufs=3) as osb_pool,
            tc.tile_pool(name="psS", bufs=2, space="PSUM") as psS,
            tc.tile_pool(name="psW", bufs=2, space="PSUM") as psW,
        ):
            # ---- tiny bias-row DMAs first (sync queue), then the batch-0/1
            # x tiles; the big weight DMAs stream on the gpsimd SWDGE queue ----
            brow_v = cpool.tile([1, D], F32)
            nc.sync.dma_start(brow_v[:], bqkv_ext[None, 2 * D : 3 * D])
            brow_fc = cpool.tile([1, D], F32)
            nc.sync.dma_start(brow_fc[:], bfc_ext[None, :])

            # ---- HAM warmup: ~10 junk matmuls (no DMA dependency) so the PE
            # clock gate opens during the DMA-bound startup window ----
            junk = cpool.tile([1, 128], BF16)
            nc.vector.memset(junk[:], 0.5)
            pwarm = psW.tile([128, 128], F32, tag="psW", name="pwarm")
            for _ in range(20):
                nc.tensor.matmul(
                    pwarm[:, :],
                    lhsT=junk[0:1, :],
                    rhs=junk[0:1, :],
                    start=True,
                    stop=True,
                )

            # ---- constants ----
            identity = cpool.tile([128, 128], F32)
            make_identity(nc, identity[:])
            ones = cpool.tile([1, 128], F32)
            nc.vector.memset(ones[:], 1.0)

            b_qk = cpool.tile([128, H], F32)  # per-partition bias for qkT tiles

            # ---- batch 0/1 x DMAs, split across sync+scalar HWDGE queues ----
            x_t, xT_t, qkT_t = {}, {}, {}

            def emit_x_dma(b, split=False):
                x_t[b] = x_pool.tile([128, 5 * D], F32, tag="x", name="x_all")
                for si, (s0, psl) in enumerate(S_TILES):
                    eng = nc.scalar if (split and si % 2) else nc.sync
                    eng.dma_start(
                        x_t[b][0:psl, si * D : (si + 1) * D],
                        x_ext[b, s0 : s0 + psl, :],
                    )

            emit_x_dma(0, split=True)
            emit_x_dma(1, split=True)
            # b_qk is a slow element-gather (strided 4B reads); queue it after
            # the startup-critical x tiles -- it is first read ~20us in
            nc.sync.dma_start(
                b_qk[:], bqkv_ext[0 : 2 * D].rearrange("(m p) -> p m", p=128)
            )

            # broadcast v/fc biases to all 128 partitions via K=1 matmul
            b_v_bc = cpool.tile([128, D], F32)
            b_fc_bc = cpool.tile([128, D], F32)
            for row, bc in ((brow_v, b_v_bc), (brow_fc, b_fc_bc)):
                pb = psW.tile([128, D], F32, tag="psW", name="pb")
                for c0, cl in CH_D:
                    nc.tensor.matmul(
                        pb[:, c0 : c0 + cl],
                        lhsT=ones[0:1, 0:128],
                        rhs=row[0:1, c0 : c0 + cl],
                        start=True,
                        stop=True,
                    )
                nc.vector.tensor_copy(bc[:], pb[:])

            # weights -> SBUF bf16 via gpsimd casting DMA; one tile per
            # contraction block k so the first qkT matmul only waits on block 0
            w_qkv_k = [
                cpool.tile([128, 3 * D], BF16, name=f"wqkv{k}") for k in range(NKT)
            ]
            w_fc_k = [cpool.tile([128, D], BF16, name=f"wfc{k}") for k in range(NKT)]
            for k in range(NKT):
                nc.gpsimd.dma_start(
                    w_qkv_k[k][:], wqkv_ext[k * 128 : (k + 1) * 128, :]
                )

            # ---- per-batch emission helpers ----

            def emit_xT_dk(b, dk):
                if b not in xT_t:
                    xT_t[b] = {}
                xT_t[b][dk] = xT_pool.tile(
                    [128, S], BF16, tag=f"xT{dk}", name=f"xT{dk}"
                )
                px = psW.tile([128, S], F32, tag="psW", name="px")
                for si, (s0, psl) in enumerate(S_TILES):
                    nc.tensor.transpose(
                        px[:, s0 : s0 + psl],
                        x_t[b][0:psl, si * D + dk * 128 : si * D + (dk + 1) * 128],
                        identity[0:psl, 0:psl],
                    )
                nc.vector.tensor_copy(xT_t[b][dk][:], px[:])

            def emit_qkT_mtile(b, m):
                # qkT tile m holds rows [m*128, (m+1)*128) = q or k of 2 heads
                if b not in qkT_t:
                    qkT_t[b] = {}
                if m in qkT_t[b]:
                    return
                qkT_t[b][m] = qkT_pool.tile(
                    [128, S], BF16, tag=f"qkT{m}", name=f"qkT{m}"
                )
                pqk = psW.tile([128, S], F32, tag="psW", name="pqk")
                for k in range(NKT):
                    for c0, cl in CH_S:
                        nc.tensor.matmul(
                            pqk[:, c0 : c0 + cl],
                            lhsT=w_qkv_k[k][:, m * 128 : (m + 1) * 128],
                            rhs=xT_t[b][k][:, c0 : c0 + cl],
                            start=(k == 0),
                            stop=(k == NKT - 1),
                        )
                # per-partition bias add on DVE (keeps ScalarE free for exp)
                nc.vector.tensor_scalar_add(
                    qkT_t[b][m][:], pqk[:], b_qk[:, m : m + 1]
                )

            v_t = {}
            v_done = set()

            def emit_v_si(b, si):
                # v natural [577, 768] + per-head ones column per si-tile
                if (b, si) in v_done:
                    return
                v_done.add((b, si))
                if b not in v_t:
                    v_all = v_pool.tile(
                        [128, 5 * H * (DH + 1)], BF16, tag="v", name="v_all"
                    )
                    v4 = v_all[:].rearrange("p (s h e) -> p s h e", s=5, h=H)
                    nc.vector.memset(v4[:, :, :, DH : DH + 1], 1.0)
                    v_t[b] = v_all
                v_all = v_t[b]
                v4 = v_all[:].rearrange("p (s h e) -> p s h e", s=5, h=H)
                s0, psl = S_TILES[si]
                pv = psW.tile([128, D], F32, tag="psW", name="pv")
                for k in range(NKT):
                    for c0, cl in CH_D:
                        nc.tensor.matmul(
                            pv[0:psl, c0 : c0 + cl],
                            lhsT=xT_t[b][k][:, s0 : s0 + psl],
                            rhs=w_qkv_k[k][:, 2 * D + c0 : 2 * D + c0 + cl],
                            start=(k == 0),
                            stop=(k == NKT - 1),
                        )
                nc.vector.tensor_add(
                    v4[0:psl, si, :, 0:DH],
                    pv[0:psl, :].rearrange("p (h e) -> p h e", h=H),
                    b_v_bc[0:psl, :].rearrange("p (h e) -> p h e", h=H),
                )

            def emit_scores_si(b, p, si, expT):
                heads = (2 * p, 2 * p + 1)
                s0, psl = S_TILES[si]
                psc = {}
                for h in heads:
                    psc[h] = psS.tile([128, S], F32, tag="psS", name=f"psc{h % 2}")
                for c0, cl in CH_S:
                    for h in heads:
                        hoff = (h % 2) * 64
                        qm, km = h // 2, NKT + h // 2
                        nc.tensor.matmul(
                            psc[h][0:psl, c0 : c0 + cl],
                            lhsT=qkT_t[b][km][hoff : hoff + 64, s0 : s0 + psl],
                            rhs=qkT_t[b][qm][hoff : hoff + 64, c0 : c0 + cl],
                            start=True,
                            stop=True,
                        )
                for h in heads:
                    nc.scalar.activation(
                        expT[h][0:psl, si * S : (si + 1) * S],
                        psc[h][0:psl, :],
                        EXP,
                        scale=float(SCALE),
                    )

            def emit_attnv_head(b, h, attnT_all, expT):
                hoff = (h % 2) * 64
                for si in range(5):
                    emit_v_si(b, si)  # no-op unless forced (last batch)
                v_all = v_t[b]
                # attn_outT [65, 577]: rows 0:64 = out^T unnorm, row 64 = sums
                po = psW.tile([65, S], F32, tag="psW", name="po")
                for si, (s0, psl) in enumerate(S_TILES):
                    for c0, cl in CH_S:
                        nc.tensor.matmul(
                            po[:, c0 : c0 + cl],
                            lhsT=v_all[
                                0:psl,
                                si * H * (DH + 1)
                                + h * (DH + 1) : si * H * (DH + 1)
                                + (h + 1) * (DH + 1),
                            ],
                            rhs=expT[h][0:psl, si * S + c0 : si * S + c0 + cl],
                            start=(si == 0),
                            stop=(si == 4),
                        )
                # normalize: rs copy (recip needs an SBUF source -- custom-DVE
                # reciprocal reading PSUM breaks on HW), staged reciprocal,
                # gpsimd broadcast, then ONE fused multiply that is the drain
                rs = small_pool.tile([1, S], F32, tag="rs", name=f"rs{h % 2}")
                nc.vector.tensor_copy(rs[:], po[64:65, :])
                rinv = small_pool.tile([1, S], F32, tag="rinv", name=f"ri{h % 2}")
                nc.vector.reciprocal_approx_fast(rinv[:], rs[:])
                rbc = small_pool.tile([128, S], F32, tag="rbc")
                nc.gpsimd.partition_broadcast(rbc[:, :], rinv[0:1, :])
                nc.vector.tensor_mul(
                    attnT_all[hoff : hoff + 64, (h // 2) * S : (h // 2 + 1) * S],
                    po[0:64, :],
                    rbc[0:64, :],
                )
                del expT[h]

            attnT_t = {}

            def emit_fc_si(b, si):
                attnT_all = attnT_t[b]
                s0, psl = S_TILES[si]
                pf = psW.tile([128, D], F32, tag="psW", name="pf")
                for k in range(NKT):
                    for c0, cl in CH_D:
                        nc.tensor.matmul(
                            pf[0:psl, c0 : c0 + cl],
                            lhsT=attnT_all[:, k * S + s0 : k * S + s0 + psl],
                            rhs=w_fc_k[k][:, c0 : c0 + cl],
                            start=(k == 0),
                            stop=(k == NKT - 1),
                        )
                osb = osb_pool.tile([128, D], F32, tag="osb")
                nc.vector.tensor_add(osb[0:psl, :], pf[0:psl, :], b_fc_bc[0:psl, :])
                nc.sync.dma_start(out_ext[b, s0 : s0 + psl, :], osb[0:psl, :])

            # ---- prologue: batch 0 transposes + first qkT tiles run during
            # the weight DMAs ----
            for dk in range(NKT):
                emit_xT_dk(0, dk)
            emit_qkT_mtile(0, 0)
            emit_qkT_mtile(0, NKT)
            for k in range(NKT):
                nc.gpsimd.dma_start(w_fc_k[k][:], wfc_ext[k * 128 : (k + 1) * 128, :])
            for dk in range(NKT):
                emit_xT_dk(1, dk)
            for si in range(5):
                emit_v_si(0, si)
            emit_qkT_mtile(0, 1)
            emit_qkT_mtile(0, NKT + 1)

            # ---- main loop: per batch, attention with budget-paced fillers ----
            for b in range(NB):
                if b + 2 < NB:
                    emit_x_dma(b + 2)

                fillers = []  # (cost_ns, fn), emission order = priority

                def F(cost, fn, *args):
                    fillers.append((cost, (lambda a: lambda: fn(*a))(args)))

                # this batch's remaining qkT tiles (deadline: scores pair p)
                for p in range(2, NKT):
                    F(COST_QKT, emit_qkT_mtile, b, p)
                    F(COST_QKT, emit_qkT_mtile, b, NKT + p)
                if b == NB - 1:
                    # last batch: its own v tiles lead (no next batch to feed)
                    for si in range(5):
                        F(COST_V_SI, emit_v_si, b, si)
                if b >= 1:
                    for si in range(5):
                        F(COST_FC_SI, emit_fc_si, b - 1, si)
                if b + 1 < NB:
                    for m in (0, NKT, 1, NKT + 1):
                        F(COST_QKT, emit_qkT_mtile, b + 1, m)
                    if b + 1 < NB - 1:
                        for si in range(5):
                            F(COST_V_SI, emit_v_si, b + 1, si)
                if b + 2 < NB:
                    for dk in range(NKT):
                        F(COST_XT_DK, emit_xT_dk, b + 2, dk)

                # try_fill fires at si 0/2/4 of pairs 1..5 and all si of pair
                # 0: 20 calls per batch; pace the queue to drain evenly
                total_cost = sum(c for c, _ in fillers)
                credit_per_step = total_cost / 20.0
                fill_iter = iter(fillers)
                acc = [0.0]

                def try_fill():
                    acc[0] += credit_per_step
                    while acc[0] > 0:
                        item = next(fill_iter, None)
                        if item is None:
                            return
                        cost, fn = item
                        fn()
                        acc[0] -= cost

                attnT_t[b] = attnT_all = attnT_pool.tile(
                    [128, NKT * S], BF16, tag="attnT", name="attnT_all"
                )
                expT = {}
                for p in range(H // 2 + 1):
                    if p < H // 2:
                        # ensure score operands exist (normally pre-filled)
                        emit_qkT_mtile(b, p)
                        emit_qkT_mtile(b, NKT + p)
                        for h in (2 * p, 2 * p + 1):
                            expT[h] = expT_pool.tile(
                                [128, 5 * S], BF16, tag="expT", name=f"expT{h % 2}"
                            )
                        for si in range(5):
                            emit_scores_si(b, p, si, expT)
                            # weave: prev pair's attnv heads into si 1 and 3
                            if p >= 1 and si == 1:
                                emit_attnv_head(b, 2 * (p - 1), attnT_all, expT)
                            elif p >= 1 and si == 3:
                                emit_attnv_head(b, 2 * (p - 1) + 1, attnT_all, expT)
                            else:
                                try_fill()
                    else:
                        emit_attnv_head(b, 2 * (p - 1), attnT_all, expT)
                        emit_attnv_head(b, 2 * (p - 1) + 1, attnT_all, expT)
                # leftover fillers run before the next batch
                for _, f in fill_iter:
                    f()

            for si in range(5):
                emit_fc_si(NB - 1, si)

    nc.compile()
    return nc


_NC_CACHE = None


def _get_nc():
    global _NC_CACHE
    if _NC_CACHE is None:
        _NC_CACHE = build_nc()
    return _NC_CACHE


def kernel(x, w_qkv, b_qkv, w_fc, b_fc, _collect=None):
    nc = _get_nc()
    x = np.ascontiguousarray(np.asarray(x, dtype=np.float32))
    w_qkv = np.ascontiguousarray(np.asarray(w_qkv, dtype=np.float32))
    b_qkv = np.ascontiguousarray(np.asarray(b_qkv, dtype=np.float32))
    w_fc = np.ascontiguousarray(np.asarray(w_fc, dtype=np.float32))
    b_fc = np.ascontiguousarray(np.asarray(b_fc, dtype=np.float32))
    in_maps = [
        {
            "x": x[i * NB : (i + 1) * NB],
            "w_qkv": w_qkv,
            "b_qkv": b_qkv,
            "w_fc": w_fc,
            "b_fc": b_fc,
        }
        for i in range(NCORES)
    ]
    kwargs = dict(_collect) if _collect else {}
    res = run_bass_kernel_spmd(nc, in_maps, core_ids=list(range(NCORES)), **kwargs)
    out = np.concatenate([res.results[i]["out"] for i in range(NCORES)], axis=0)
    if _collect is not None and isinstance(_collect, dict):
        _collect["result"] = res
    return out.astype(np.float32)


if __name__ == "__main__":
    xs = np.random.randn(B, S, D).astype(np.float32)
    lim = 1.0 / np.sqrt(D)
    rng = np.random.default_rng(0)
    wq = rng.uniform(-lim, lim, (D, 3 * D)).astype(np.float32)
    bq = rng.uniform(-lim, lim, (3 * D,)).astype(np.float32)
    wf = rng.uniform(-lim, lim, (D, D)).astype(np.float32)
    bf = rng.uniform(-lim, lim, (D,)).astype(np.float32)
    o = kernel(xs, wq, bq, wf, bf)
    print("out", o.shape, o.dtype)


# revision 20
# speedup vs baseline: 1.0687x; 1.0687x over previous
"""Trainium2 Bass kernel: 12-head attention block (qkv proj -> softmax attn -> fc).

Reference semantics (B=32, S=577, D=768, H=12, Dh=64):
    qkv = x @ w_qkv + b_qkv
    q, k, v = split(qkv); attn = softmax(q k^T / 8) v
    out = attn @ w_fc + b_fc

Sharding: data-parallel over batch across 8 NeuronCores (4 images per core),
weights replicated, no collectives. Compute in bf16 with fp32 PSUM accumulation.

Layout strategy per core (all matmuls contract over the partition dim):
  - xT [768, 577] built from x via PE transposes (fp32 transpose mode).
  - qkT [1536, 577] = w_qkv[:, :1536]^T . xT  (w_qkv stationary in natural layout).
  - v   [577, 768]  = xT^T . w_qkv[:, 1536:]  (natural layout, per-head ones column
    appended so attention row-sums fall out of the attn@v matmul for free).
  - scoresT[sk, sq] = kT_h^T . qT_h; heads paired even/odd so their K=64
    matmuls land on disjoint PE row groups and run concurrently. exp on
    ScalarE (scale=1/8 folded into exp; no max subtraction -- scores are O(1)).
  - attn_outT[65, sq] = (v_h|1)^T . expT ; row 64 = softmax denominators.
  - normalize: reciprocal_approx_fast straight off the PSUM sum row, gpsimd
    partition_broadcast, then ONE fused DVE multiply (po x rbc -> attnT bf16)
    that is also the PSUM drain (no separate copy).
  - qkT drain is a DVE tensor_scalar_add (per-partition bias), NOT a ScalarE
    activation: ScalarE is the attention pacer (2x 774ns exp per si-step) and
    must not carry the bias adds too.
  - fc: out[s, :] = attn_T_k^T . w_fc_k (natural w_fc), + b_fc broadcast.

Scheduling (v2, trace-driven): the HAM clock gate re-throttled the PE to half
clock for ~105us in v1 (startup, batch boundaries, last batch) whenever the MM
supply thinned. Fixes:
  - ~10 junk warmup matmuls at t~0 so HAM unthrottles during the DMA-bound
    startup instead of 30us in.
  - all non-attention PE work is cut into ~1-2us filler units (qkT per m-tile,
    v per si-tile, fc per si-tile, xT per dk) and fed into the attention
    si-step loop with an even budget (queue_total / 30 steps per batch), so
    the MM stream never thins at batch boundaries.
  - attnv heads are woven INTO the next pair's scores si-loop (head A after
    si1, head B after si3) instead of running as a block after it.
  - batch 3 (which has no next batch to prefetch) gets v(3) moved into its own
    attention as its leading fillers, plus fc(2), so its MM supply matches the
    other batches.
PSUM: two 2-slot pools (scores vs everything else), 8 banks total.
"""

import os
import sys

import numpy as np

for _p in ("/opt/trn_rl_repo", "/root/.axon_site/_ro/trn_rl_repo"):
    if os.path.isdir(_p) and _p not in sys.path:
        sys.path.insert(0, _p)

import concourse.bass as bass  # noqa: E402
import concourse.tile as tile  # noqa: E402
from concourse import bacc, mybir  # noqa: E402
from concourse.bass_utils import run_bass_kernel_spmd  # noqa: E402
from concourse.masks import make_identity  # noqa: E402

F32 = mybir.dt.float32
BF16 = mybir.dt.bfloat16

B, S, D = 32, 577, 768
H, DH = 12, 64
NCORES = 8
NB = B // NCORES  # 4 batch images per core
SCALE = DH**-0.5  # 0.125
NKT = D // 128  # 6 contraction tiles of 128
S_TILES = [(0, 128), (128, 128), (256, 128), (384, 128), (512, 65)]
CH_S = [(0, 512), (512, 65)]  # 577 split at PSUM-bank boundary
CH_D = [(0, 512), (512, 256)]  # 768 split at PSUM-bank boundary
EXP = mybir.ActivationFunctionType.Exp
IDENT = mybir.ActivationFunctionType.Identity

# filler unit cost estimates (ns of PE stream time, warm)
COST_QKT = 1500
COST_V_SI = 2000
COST_FC_SI = 2000
COST_XT_DK = 900


def build_nc():
    nc = bacc.Bacc(None)
    x_ext = nc.declare_dram_parameter("x", [NB, S, D], F32, isOutput=False)
    wqkv_ext = nc.declare_dram_parameter("w_qkv", [D, 3 * D], F32, isOutput=False)
    bqkv_ext = nc.declare_dram_parameter("b_qkv", [3 * D], F32, isOutput=False)
    wfc_ext = nc.declare_dram_parameter("w_fc", [D, D], F32, isOutput=False)
    bfc_ext = nc.declare_dram_parameter("b_fc", [D], F32, isOutput=False)
    out_ext = nc.declare_dram_parameter("out", [NB, S, D], F32, isOutput=True)

    with tile.TileContext(nc) as tc:
        with (
            tc.tile_pool(name="const", bufs=1) as cpool,
            tc.tile_pool(name="x", bufs=2) as x_pool,
            tc.tile_pool(name="xT", bufs=2) as xT_pool,
            tc.tile_pool(name="qkT", bufs=2) as qkT_pool,
            tc.tile_pool(name="v", bufs=2) as v_pool,
            tc.tile_pool(name="expT", bufs=5) as expT_pool,
            tc.tile_pool(name="attnT", bufs=2) as attnT_pool,
            tc.tile_pool(name="small", bufs=3) as small_pool,
            tc.tile_pool(name="osb", bufs=2) as osb_pool,
            tc.tile_pool(name="psS", bufs=2, space="PSUM") as psS,
            tc.tile_pool(name="psW", bufs=2, space="PSUM") as psW,
        ):
            # ---- tiny bias-row DMAs first (sync queue), then the batch-0/1
            # x tiles; the big weight DMAs stream on the gpsimd SWDGE queue ----
            brow_v = cpool.tile([1, D], F32)
            nc.sync.dma_start(brow_v[:], bqkv_ext[None, 2 * D : 3 * D])
            brow_fc = cpool.tile([1, D], F32)
            nc.sync.dma_start(brow_fc[:], bfc_ext[None, :])

            # ---- HAM warmup: ~10 junk matmuls (no DMA dependency) so the PE
            # clock gate opens during the DMA-bound startup window ----
            # N=512 keeps the PE duty cycle ~100% (the N=128 variant spends
            # half its time in LDWEIGHTS, which HAM does not count as busy)
            junk = cpool.tile([1, 512], BF16)
            nc.vector.memset(junk[:], 0.5)
            pwarm = psW.tile([128, 512], F32, tag="psW", name="pwarm")
            for _ in range(10):
                nc.tensor.matmul(
                    pwarm[:, :],
                    lhsT=junk[0:1, 0:128],
                    rhs=junk[0:1, :],
                    start=True,
                    stop=True,
                )

            # ---- constants ----
            identity = cpool.tile([128, 128], F32)
            make_identity(nc, identity[:])
            ones = cpool.tile([1, 128], F32)
            nc.vector.memset(ones[:], 1.0)

            b_qk = cpool.tile([128, H], F32)  # per-partition bias for qkT tiles

            # ---- batch 0/1 x DMAs, split across sync+scalar HWDGE queues ----
            x_t, xT_t, qkT_t = {}, {}, {}

            def emit_x_dma(b, split=False):
                x_t[b] = x_pool.tile([128, 5 * D], F32, tag="x", name="x_all")
                for si, (s0, psl) in enumerate(S_TILES):
                    eng = nc.scalar if (split and si % 2) else nc.sync
                    eng.dma_start(
                        x_t[b][0:psl, si * D : (si + 1) * D],
                        x_ext[b, s0 : s0 + psl, :],
                    )

            emit_x_dma(0, split=True)
            emit_x_dma(1, split=True)
            # b_qk is a slow element-gather (strided 4B reads); queue it after
            # the startup-critical x tiles -- it is first read ~20us in
            nc.sync.dma_start(
                b_qk[:], bqkv_ext[0 : 2 * D].rearrange("(m p) -> p m", p=128)
            )

            # broadcast v/fc biases to all 128 partitions via K=1 matmul
            b_v_bc = cpool.tile([128, D], F32)
            b_fc_bc = cpool.tile([128, D], F32)
            for row, bc in ((brow_v, b_v_bc), (brow_fc, b_fc_bc)):
                pb = psW.tile([128, D], F32, tag="psW", name="pb")
                for c0, cl in CH_D:
                    nc.tensor.matmul(
                        pb[:, c0 : c0 + cl],
                        lhsT=ones[0:1, 0:128],
                        rhs=row[0:1, c0 : c0 + cl],
                        start=True,
                        stop=True,
                    )
                nc.vector.tensor_copy(bc[:], pb[:])

            # weights -> SBUF bf16 via gpsimd casting DMA, column-prioritized:
            # scores pair p needs m-tiles (p, 6+p); ship m0/m6/m1/m7 first
            # (~0.4MB, lands ~3us), then the v columns (needed ~14us in), then
            # the remaining q/k columns in deadline order
            w_qkv_k = [
                cpool.tile([128, 3 * D], BF16, name=f"wqkv{k}") for k in range(NKT)
            ]
            w_fc_k = [cpool.tile([128, D], BF16, name=f"wfc{k}") for k in range(NKT)]
            WCOLS = [
                (0, 128),  # m0
                (768, 128),  # m6
                (128, 128),  # m1
                (896, 128),  # m7
                (1536, 768),  # v
                (256, 512),  # m2-5
                (1024, 512),  # m8-11
            ]
            for c0, cl in WCOLS:
                for k in range(NKT):
                    nc.gpsimd.dma_start(
                        w_qkv_k[k][:, c0 : c0 + cl],
                        wqkv_ext[k * 128 : (k + 1) * 128, c0 : c0 + cl],
                    )

            # ---- per-batch emission helpers ----

            def emit_xT_dk(b, dk):
                if b not in xT_t:
                    xT_t[b] = {}
                xT_t[b][dk] = xT_pool.tile(
                    [128, S], BF16, tag=f"xT{dk}", name=f"xT{dk}"
                )
                px = psW.tile([128, S], F32, tag="psW", name="px")
                for si, (s0, psl) in enumerate(S_TILES):
                    nc.tensor.transpose(
                        px[:, s0 : s0 + psl],
                        x_t[b][0:psl, si * D + dk * 128 : si * D + (dk + 1) * 128],
                        identity[0:psl, 0:psl],
                    )
                nc.vector.tensor_copy(xT_t[b][dk][:], px[:])

            def emit_qkT_mtile(b, m):
                # qkT tile m holds rows [m*128, (m+1)*128) = q or k of 2 heads
                if b not in qkT_t:
                    qkT_t[b] = {}
                if m in qkT_t[b]:
                    return
                qkT_t[b][m] = qkT_pool.tile(
                    [128, S], BF16, tag=f"qkT{m}", name=f"qkT{m}"
                )
                pqk = psW.tile([128, S], F32, tag="psW", name="pqk")
                for k in range(NKT):
                    for c0, cl in CH_S:
                        nc.tensor.matmul(
                            pqk[:, c0 : c0 + cl],
                            lhsT=w_qkv_k[k][:, m * 128 : (m + 1) * 128],
                            rhs=xT_t[b][k][:, c0 : c0 + cl],
                            start=(k == 0),
                            stop=(k == NKT - 1),
                        )
                # per-partition bias add on ScalarE: DVE (242us) is hotter than
                # ScalarE (172us), and ScalarE has headroom under the exp pace
                nc.scalar.activation(
                    qkT_t[b][m][:], pqk[:], IDENT, bias=b_qk[:, m : m + 1]
                )

            v_t = {}
            v_done = set()

            def emit_v_si(b, si):
                # v natural [577, 768] + per-head ones column per si-tile
                if (b, si) in v_done:
                    return
                v_done.add((b, si))
                if b not in v_t:
                    v_all = v_pool.tile(
                        [128, 5 * H * (DH + 1)], BF16, tag="v", name="v_all"
                    )
                    v4 = v_all[:].rearrange("p (s h e) -> p s h e", s=5, h=H)
                    nc.vector.memset(v4[:, :, :, DH : DH + 1], 1.0)
                    v_t[b] = v_all
                v_all = v_t[b]
                v4 = v_all[:].rearrange("p (s h e) -> p s h e", s=5, h=H)
                s0, psl = S_TILES[si]
                pv = psW.tile([128, D], F32, tag="psW", name="pv")
                for k in range(NKT):
                    for c0, cl in CH_D:
                        nc.tensor.matmul(
                            pv[0:psl, c0 : c0 + cl],
                            lhsT=xT_t[b][k][:, s0 : s0 + psl],
                            rhs=w_qkv_k[k][:, 2 * D + c0 : 2 * D + c0 + cl],
                            start=(k == 0),
                            stop=(k == NKT - 1),
                        )
                nc.vector.tensor_add(
                    v4[0:psl, si, :, 0:DH],
                    pv[0:psl, :].rearrange("p (h e) -> p h e", h=H),
                    b_v_bc[0:psl, :].rearrange("p (h e) -> p h e", h=H),
                )

            def emit_scores_si(b, p, si, expT):
                heads = (2 * p, 2 * p + 1)
                s0, psl = S_TILES[si]
                psc = {}
                for h in heads:
                    psc[h] = psS.tile([128, S], F32, tag="psS", name=f"psc{h % 2}")
                for c0, cl in CH_S:
                    for h in heads:
                        hoff = (h % 2) * 64
                        qm, km = h // 2, NKT + h // 2
                        nc.tensor.matmul(
                            psc[h][0:psl, c0 : c0 + cl],
                            lhsT=qkT_t[b][km][hoff : hoff + 64, s0 : s0 + psl],
                            rhs=qkT_t[b][qm][hoff : hoff + 64, c0 : c0 + cl],
                            start=True,
                            stop=True,
                        )
                for h in heads:
                    nc.scalar.activation(
                        expT[h][0:psl, si * S : (si + 1) * S],
                        psc[h][0:psl, :],
                        EXP,
                        scale=float(SCALE),
                    )

            def emit_attnv_head(b, h, attnT_all, expT):
                hoff = (h % 2) * 64
                for si in range(5):
                    emit_v_si(b, si)  # no-op unless forced (last batch)
                v_all = v_t[b]
                # attn_outT [65, 577]: rows 0:64 = out^T unnorm, row 64 = sums
                po = psW.tile([65, S], F32, tag="psW", name="po")
                for si, (s0, psl) in enumerate(S_TILES):
                    for c0, cl in CH_S:
                        nc.tensor.matmul(
                            po[:, c0 : c0 + cl],
                            lhsT=v_all[
                                0:psl,
                                si * H * (DH + 1)
                                + h * (DH + 1) : si * H * (DH + 1)
                                + (h + 1) * (DH + 1),
                            ],
                            rhs=expT[h][0:psl, si * S + c0 : si * S + c0 + cl],
                            start=(si == 0),
                            stop=(si == 4),
                        )
                # normalize: drain po fast via copy (fusing the mul with the
                # PSUM read extends po occupancy and stalls the psW ring --
                # measured slower), staged reciprocal (SBUF source: custom-DVE
                # recip reading PSUM breaks on HW), broadcast, in-place mul
                nc.vector.tensor_copy(
                    attnT_all[hoff : hoff + 64, (h // 2) * S : (h // 2 + 1) * S],
                    po[0:64, :],
                )
                rs = small_pool.tile([1, S], F32, tag="rs", name=f"rs{h % 2}")
                nc.vector.tensor_copy(rs[:], po[64:65, :])
                rinv = small_pool.tile([1, S], F32, tag="rinv", name=f"ri{h % 2}")
                nc.vector.reciprocal_approx_fast(rinv[:], rs[:])
                rbc = small_pool.tile([128, S], F32, tag="rbc")
                nc.gpsimd.partition_broadcast(rbc[:, :], rinv[0:1, :])
                nc.vector.tensor_mul(
                    attnT_all[hoff : hoff + 64, (h // 2) * S : (h // 2 + 1) * S],
                    attnT_all[hoff : hoff + 64, (h // 2) * S : (h // 2 + 1) * S],
                    rbc[hoff : hoff + 64, :],
                )
                del expT[h]

            attnT_t = {}

            def emit_fc_si(b, si):
                attnT_all = attnT_t[b]
                s0, psl = S_TILES[si]
                pf = psW.tile([128, D], F32, tag="psW", name="pf")
                for k in range(NKT):
                    for c0, cl in CH_D:
                        nc.tensor.matmul(
                            pf[0:psl, c0 : c0 + cl],
                            lhsT=attnT_all[:, k * S + s0 : k * S + s0 + psl],
                            rhs=w_fc_k[k][:, c0 : c0 + cl],
                            start=(k == 0),
                            stop=(k == NKT - 1),
                        )
                osb = osb_pool.tile([128, D], F32, tag="osb")
                nc.vector.tensor_add(osb[0:psl, :], pf[0:psl, :], b_fc_bc[0:psl, :])
                eng = nc.scalar if si % 2 else nc.sync
                eng.dma_start(out_ext[b, s0 : s0 + psl, :], osb[0:psl, :])

            # ---- prologue: batch 0/1 transposes + scores-pair-0/1 qkT tiles
            # only; v(0) goes into batch 0's filler queue so the first scores
            # are not stuck behind v-column-gated matmuls in the PE FIFO ----
            for dk in range(NKT):
                emit_xT_dk(0, dk)
            emit_qkT_mtile(0, 0)
            emit_qkT_mtile(0, NKT)
            for dk in range(NKT):
                emit_xT_dk(1, dk)
            emit_qkT_mtile(0, 1)
            emit_qkT_mtile(0, NKT + 1)
            for k in range(NKT):
                nc.gpsimd.dma_start(w_fc_k[k][:], wfc_ext[k * 128 : (k + 1) * 128, :])

            # ---- main loop: per batch, attention with budget-paced fillers ----
            for b in range(NB):
                if b + 2 < NB:
                    emit_x_dma(b + 2)

                fillers = []  # (cost_ns, fn), emission order = priority

                def F(cost, fn, *args):
                    fillers.append((cost, (lambda a: lambda: fn(*a))(args)))

                if b == 0 or b == NB - 1:
                    # batch 0: v(0) runs as fillers (its weight columns land
                    # ~12us in); last batch: no next batch to feed
                    for si in range(5):
                        F(COST_V_SI, emit_v_si, b, si)
                # this batch's remaining qkT tiles (deadline: scores pair p)
                for p in range(2, NKT):
                    F(COST_QKT, emit_qkT_mtile, b, p)
                    F(COST_QKT, emit_qkT_mtile, b, NKT + p)
                if b >= 1:
                    for si in range(5):
                        F(COST_FC_SI, emit_fc_si, b - 1, si)
                if b + 1 < NB:
                    for m in (0, NKT, 1, NKT + 1):
                        F(COST_QKT, emit_qkT_mtile, b + 1, m)
                    if b + 1 < NB - 1:
                        for si in range(5):
                            F(COST_V_SI, emit_v_si, b + 1, si)
                if b + 2 < NB:
                    for dk in range(NKT):
                        F(COST_XT_DK, emit_xT_dk, b + 2, dk)

                # try_fill fires at si 0/2/4 of pairs 1..5 and all si of pair
                # 0: 20 calls per batch; pace the queue to drain evenly
                total_cost = sum(c for c, _ in fillers)
                credit_per_step = total_cost / 20.0
                fill_iter = iter(fillers)
                acc = [0.0]

                def try_fill():
                    acc[0] += credit_per_step
                    while acc[0] > 0:
                        item = next(fill_iter, None)
                        if item is None:
                            return
                        cost, fn = item
                        fn()
                        acc[0] -= cost

                attnT_t[b] = attnT_all = attnT_pool.tile(
                    [128, NKT * S], BF16, tag="attnT", name="attnT_all"
                )
                expT = {}
                for p in range(H // 2 + 1):
                    if p < H // 2:
                        # ensure score operands exist (normally pre-filled)
                        emit_qkT_mtile(b, p)
                        emit_qkT_mtile(b, NKT + p)
                        for h in (2 * p, 2 * p + 1):
                            expT[h] = expT_pool.tile(
                                [128, 5 * S], BF16, tag="expT", name=f"expT{h % 2}"
                            )
                        for si in range(5):
                            emit_scores_si(b, p, si, expT)
                            # weave: prev pair's attnv heads into si 1 and 3
                            if p >= 1 and si == 1:
                                emit_attnv_head(b, 2 * (p - 1), attnT_all, expT)
                            elif p >= 1 and si == 3:
                                emit_attnv_head(b, 2 * (p - 1) + 1, attnT_all, expT)
                            else:
                                try_fill()
                    else:
                        emit_attnv_head(b, 2 * (p - 1), attnT_all, expT)
                        emit_attnv_head(b, 2 * (p - 1) + 1, attnT_all, expT)
                # leftover fillers run before the next batch
                for _, f in fill_iter:
                    f()

            for si in range(5):
                emit_fc_si(NB - 1, si)

    nc.compile()
    return nc


_NC_CACHE = None


def _get_nc():
    global _NC_CACHE
    if _NC_CACHE is None:
        _NC_CACHE = build_nc()
    return _NC_CACHE


def kernel(x, w_qkv, b_qkv, w_fc, b_fc, _collect=None):
    nc = _get_nc()
    x = np.ascontiguousarray(np.asarray(x, dtype=np.float32))
    w_qkv = np.ascontiguousarray(np.asarray(w_qkv, dtype=np.float32))
    b_qkv = np.ascontiguousarray(np.asarray(b_qkv, dtype=np.float32))
    w_fc = np.ascontiguousarray(np.asarray(w_fc, dtype=np.float32))
    b_fc = np.ascontiguousarray(np.asarray(b_fc, dtype=np.float32))
    in_maps = [
        {
            "x": x[i * NB : (i + 1) * NB],
            "w_qkv": w_qkv,
            "b_qkv": b_qkv,
            "w_fc": w_fc,
            "b_fc": b_fc,
        }
        for i in range(NCORES)
    ]
    kwargs = dict(_collect) if _collect else {}
    res = run_bass_kernel_spmd(nc, in_maps, core_ids=list(range(NCORES)), **kwargs)
    out = np.concatenate([res.results[i]["out"] for i in range(NCORES)], axis=0)
    if _collect is not None and isinstance(_collect, dict):
        _collect["result"] = res
    return out.astype(np.float32)


if __name__ == "__main__":
    xs = np.random.randn(B, S, D).astype(np.float32)
    lim = 1.0 / np.sqrt(D)
    rng = np.random.default_rng(0)
    wq = rng.uniform(-lim, lim, (D, 3 * D)).astype(np.float32)
    bq = rng.uniform(-lim, lim, (3 * D,)).astype(np.float32)
    wf = rng.uniform(-lim, lim, (D, D)).astype(np.float32)
    bf = rng.uniform(-lim, lim, (D,)).astype(np.float32)
    o = kernel(xs, wq, bq, wf, bf)
    print("out", o.shape, o.dtype)
